# revision 23
# baseline (speedup 1.0000x reference)
"""DisentangleLossBatch Trainium2 kernel (8 NeuronCores, data-parallel).

Math: loss = sum|mean_b(G[idx_g(b), idx_h(b)]) - I| over the 8x8 top-k
Gram matrix, G = Cn @ Cn.T, idx = top-8 indices of each token's 512 pose
logits.  G[i_g,i_h] = <Cn[i_g,:], Cn[i_h,:]> = <R_g, R_h>.

This version never materializes G and never gathers from HBM (the
per-element indirect-DMA drain rate of ~4ns/element made a gather-based
kernel descriptor-bound at ~460us/core).  Instead everything stays on
the PE array:

  * per tile, the flattened top-8 index row (1024 = 128 tokens x 8
    slots, f16) is broadcast to all partitions with a rank-1 matmul
    (ones[1,128]^T x idx[1,1024] -> PSUM), and 4 is_equal compares
    against per-partition iota columns build the one-hot matrix
    M[d-chunk, (tok,slot)] in f16 (DVE+Pool split).
  * RT = Cn^T @ M  via 8 PE matmuls/tile -> RT[e, (tok,slot)] in PSUM;
    Act engine copies to f16 SBUF.  RT column (tok,slot) is the
    selected codebook row - the "gather" is a matmul against one-hots.
  * pair dots: for each 16-token group, PE computes RT_grp^T @ RT_grp
    [128,128], PSUM-accumulated over ALL groups/tiles/e-chunks in one
    512-matmul chain.  Its 16 diagonal 8x8 blocks hold sum_b R_g.R_h;
    off-diagonal blocks are cross-token garbage that the host ignores.
  * each core ships the single [128,128] f32 accumulator; the host sums
    diagonal blocks across cores, |.|, scales - the unshard step.

Engine budget per core: PE ~150-190us (critical), DVE ~85us (top8 +
half the compares), Pool ~35us, Act ~80us (PSUM evictions), DMA ~30us.
"""
import sys
import numpy as np

for _p in ("/opt/trn_rl_repo",):
    if _p not in sys.path:
        sys.path.insert(0, _p)

from contextlib import ExitStack

import concourse.bass as bass
import concourse.bacc as bacc
import concourse.tile as tile
import concourse.mybir as mybir
from concourse.bass_utils import run_bass_kernel_spmd

P = 128
N_CORES = 8
B, N, D, E = 32, 1024, 512, 256
G8 = 8
BN = B * N                       # 32768 tokens
BN_PER_CORE = BN // N_CORES      # 4096
T = BN_PER_CORE // P             # 32 tiles per core
NCOL = P * G8                    # 1024 one-hot columns per tile
NGRP = P // 16                   # 8 16-token dot groups per tile
GT = 8                           # tiles handled by the DMA-gather path
CH = 2                           # tiles per gather chunk
GC = GT // CH                    # gather chunks
NPAIR = (G8 * (G8 - 1)) // 2     # 28
CHN = P * CH * NPAIR             # 7168 descriptors per chunk
f32 = mybir.dt.float32
f16 = mybir.dt.float16
i32 = mybir.dt.int32
u32 = mybir.dt.uint32

# diagonal slot layout: for d = 1..7, slots [DIAG_OFF[d], +8-d) are pairs
# (g, g+d)
DIAG_OFF = {}
_off = 0
for _d in range(1, G8):
    DIAG_OFF[_d] = _off
    _off += G8 - _d
assert _off == NPAIR
PAIRS = [(g, g + d) for d in range(1, G8) for g in range(G8 - d)]


def build_nc(debug=False):
    nc = bacc.Bacc("TRN2", target_bir_lowering=False, debug=False,
                   num_devices=N_CORES)
    pose = nc.dram_tensor("pose", [BN_PER_CORE, D], f32, kind="ExternalInput")
    cb = nc.dram_tensor("codebook", [D, E], f32, kind="ExternalInput")
    iota4 = nc.dram_tensor("iota4", [P, 4], f32, kind="ExternalInput")
    ones1 = nc.dram_tensor("ones1", [1, P], f16, kind="ExternalInput")
    ident = nc.dram_tensor("ident", [P, P], f32, kind="ExternalInput")
    dot_out = nc.dram_tensor("dot_out", [P, P], f32, kind="ExternalOutput")
    acc_out = nc.dram_tensor("acc_out", [GC, CHN], f32, kind="ExternalOutput")
    g_hbm = nc.dram_tensor("g_scratch", [D * D + D], f32)

    with tile.TileContext(nc) as tc, ExitStack() as ctx:
        const_pool = ctx.enter_context(tc.tile_pool(name="const", bufs=1))
        prep_pool = ctx.enter_context(tc.tile_pool(name="prep", bufs=1))
        in_pool = ctx.enter_context(tc.tile_pool(name="in", bufs=6))
        small_pool = ctx.enter_context(tc.tile_pool(name="small", bufs=4))
        m_pool = ctx.enter_context(tc.tile_pool(name="m", bufs=4))
        rt_pool = ctx.enter_context(tc.tile_pool(name="rt", bufs=4))
        bc_pool = ctx.enter_context(tc.tile_pool(name="bc", bufs=1, space="PSUM"))
        rtps_pool = ctx.enter_context(tc.tile_pool(name="rtps", bufs=3, space="PSUM"))
        dot_pool = ctx.enter_context(tc.tile_pool(name="dot", bufs=1, space="PSUM"))

        # ---- constants ----
        iota16_sb = const_pool.tile([P, 4], f32)
        nc.sync.dma_start(iota16_sb[:], iota4.ap())
        ones_sb = const_pool.tile([1, P], f16)
        nc.sync.dma_start(ones_sb[:], ones1.ap())
        ident_sb = const_pool.tile([P, P], f32)
        nc.sync.dma_start(ident_sb[:], ident.ap())

        # ---- codebook -> normalized rows, f16, [d-chunk, e] ----
        cb_sb = prep_pool.tile([P, 4, E], f32)
        cb_v = cb.ap().rearrange("(k p) e -> k p e", p=P)
        for k in range(4):
            nc.sync.dma_start(cb_sb[:, k, :], cb_v[k])
        sq = prep_pool.tile([P, E], f32)
        nrm2 = prep_pool.tile([P, 4], f32)
        for k in range(4):
            nc.scalar.activation(sq[:], cb_sb[:, k, :],
                                 mybir.ActivationFunctionType.Square,
                                 accum_out=nrm2[:, k:k + 1])
        nrm = prep_pool.tile([P, 4], f32)
        nc.scalar.sqrt(nrm[:], nrm2[:])
        rnorm = prep_pool.tile([P, 4], f32)
        nc.vector.reciprocal(rnorm[:], nrm[:])
        cn16 = prep_pool.tile([P, 4, E], f16)
        for k in range(4):
            nc.scalar.activation(cn16[:, k, :], cb_sb[:, k, :],
                                 mybir.ActivationFunctionType.Copy,
                                 scale=rnorm[:, k:k + 1])

        # f32 normalized rows + Gram table in HBM for the gather path
        cn = prep_pool.tile([P, 4, E], f32)
        for k in range(4):
            nc.scalar.activation(cn[:, k, :], cb_sb[:, k, :],
                                 mybir.ActivationFunctionType.Copy,
                                 scale=rnorm[:, k:k + 1])
        cnT = prep_pool.tile([P, 2, D], f32)
        for k in range(4):
            for j in range(2):
                ps_t = bc_pool.tile([P, P], f32)
                nc.tensor.transpose(ps_t[:], cn[:, k, j * P:(j + 1) * P],
                                    ident_sb[:])
                nc.scalar.copy(cnT[:, j, k * P:(k + 1) * P], ps_t[:])
        g_row_sb = prep_pool.tile([P, 4, D], f32)
        g_v = g_hbm.ap().rearrange("(r c) -> r c", c=D)
        for m in range(4):
            ps_g = bc_pool.tile([P, D], f32)
            for j in range(2):
                nc.tensor.matmul(ps_g[:], lhsT=cnT[:, j, m * P:(m + 1) * P],
                                 rhs=cnT[:, j, :],
                                 start=(j == 0), stop=(j == 1))
            nc.scalar.copy(g_row_sb[:, m, :], ps_g[:])
            nc.sync.dma_start(g_v[m * P:(m + 1) * P, :], g_row_sb[:, m, :])

        idx_all = prep_pool.tile([P, T, G8], u32)
        idx_f = prep_pool.tile([P, T, G8], f16)
        flat = prep_pool.tile([1, T, NCOL], f16)
        dot_ps = dot_pool.tile([P, P], f32)
        pose_v = pose.ap().rearrange("(t p) d -> t p d", p=P)
        n_dot = (T - GT) * 2 * NGRP

        idx_f32 = prep_pool.tile([P, GT, G8], f32)
        pidx_i = prep_pool.tile([P, GT, NPAIR], i32)
        acc = prep_pool.tile([GC, CHN, 1], f32)

        # ---- DMA-gather path: tiles 0..GT-1 ----
        for c in range(GC):
            c0 = c * CH
            sl_t = slice(c0, c0 + CH)
            for t in range(c0, c0 + CH):
                pt = in_pool.tile([P, D], f32)
                nc.sync.dma_start(pt[:], pose_v[t])
                mx = small_pool.tile([P, G8], f32)
                nc.vector.max(mx[:], pt[:])
                nc.vector.max_index(idx_all[:, t, :], mx[:], pt[:])
            nc.vector.tensor_copy(idx_f32[:, sl_t, :], idx_all[:, sl_t, :])
            for dd in range(1, G8):
                o, w = DIAG_OFF[dd], G8 - dd
                nc.vector.scalar_tensor_tensor(
                    pidx_i[:, sl_t, o:o + w],
                    idx_f32[:, sl_t, 0:w], float(D),
                    idx_f32[:, sl_t, dd:G8],
                    op0=mybir.AluOpType.mult, op1=mybir.AluOpType.add)
            nc.gpsimd.indirect_dma_start(
                out=acc[c:c + 1, :, :],
                out_offset=None,
                in_=g_hbm.ap().rearrange("(a b) -> a b", b=1),
                in_offset=bass.IndirectOffsetOnAxis(
                    ap=pidx_i[:, sl_t, :].rearrange("p a b -> p (a b)"),
                    axis=0),
            )

        # ---- PE path: tiles GT..T-1 ----
        m_tiles = [None] * T
        rt_tiles = [None] * T

        def front(t):
            pt = in_pool.tile([P, D], f32)
            nc.sync.dma_start(pt[:], pose_v[t])
            mx = small_pool.tile([P, G8], f32)
            nc.vector.max(mx[:], pt[:])
            nc.vector.max_index(idx_all[:, t, :], mx[:], pt[:])
            nc.vector.tensor_copy(idx_f[:, t, :], idx_all[:, t, :])
            # flatten [128,8] -> [1,1024]  (col = tok*8 + slot)
            nc.sync.dma_start(flat[:, t, :], idx_f[:, t, :])

            # broadcast to all partitions: ones[1,128]^T x flat[1,1024]
            # (psum matmul outputs are capped at one bank = 512 f32);
            # Act evicts to f16 so the compares run 16-bit on DVE
            m_sb = m_pool.tile([P, 4, NCOL], f16)
            bcf = m_pool.tile([P, NCOL], f16)
            for nh in range(2):
                sl_n = slice(nh * 512, (nh + 1) * 512)
                bc_ps = bc_pool.tile([P, 512], f32)
                nc.tensor.matmul(bc_ps[:], lhsT=ones_sb[:],
                                 rhs=flat[:, t, sl_n],
                                 start=True, stop=True)
                nc.scalar.copy(bcf[:, sl_n], bc_ps[:])
            for k in range(4):
                nc.vector.tensor_scalar(
                    m_sb[:, k, :], bcf[:],
                    iota16_sb[:, k:k + 1], None,
                    op0=mybir.AluOpType.is_equal)
            m_tiles[t] = m_sb

        def mid(t):
            # RT[e, col] = Cn^T M : 2 e-chunks x 2 col-halves x 4 d-chunks
            m_sb = m_tiles[t]
            rt_sb = rt_pool.tile([P, 2, NCOL], f16)
            for ec in range(2):
                for nh in range(2):
                    sl_n = slice(nh * 512, (nh + 1) * 512)
                    rt_ps = rtps_pool.tile([P, 512], f32)
                    for k in range(4):
                        nc.tensor.matmul(
                            rt_ps[:],
                            lhsT=cn16[:, k, ec * P:(ec + 1) * P],
                            rhs=m_sb[:, k, sl_n],
                            start=(k == 0), stop=(k == 3))
                    nc.scalar.copy(rt_sb[:, ec, sl_n], rt_ps[:])
            rt_tiles[t] = rt_sb

        def back(t):
            # pair dots: RT_grp^T RT_grp accumulated into one [128,128]
            rt_sb = rt_tiles[t]
            for g in range(NGRP):
                sl = slice(g * P, (g + 1) * P)
                for ec in range(2):
                    nc.tensor.matmul(dot_ps[:],
                                     lhsT=rt_sb[:, ec, sl],
                                     rhs=rt_sb[:, ec, sl],
                                     start=(back.di == 0),
                                     stop=(back.di == n_dot - 1))
                    back.di += 1
        back.di = 0

        for t in range(GT, T + 2):
            if t < T:
                front(t)
            if GT + 1 <= t <= T:
                mid(t - 1)
            if t >= GT + 2:
                back(t - 2)

        out_sb = prep_pool.tile([P, P], f32)
        nc.scalar.copy(out_sb[:], dot_ps[:])
        nc.sync.dma_start(dot_out.ap(), out_sb[:])
        nc.sync.dma_start(acc_out.ap(), acc[:, :, 0])

    nc.compile()
    return nc


_NC_CACHE = None


def _get_nc():
    global _NC_CACHE
    if _NC_CACHE is None:
        _NC_CACHE = build_nc()
    return _NC_CACHE


def make_in_maps(pose_code: np.ndarray, codebook: np.ndarray):
    flat = np.ascontiguousarray(
        pose_code.reshape(BN, D).astype(np.float32, copy=False))
    cbf = np.ascontiguousarray(codebook.astype(np.float32, copy=False))
    iota4 = (np.arange(P)[:, None] + 128 * np.arange(4)[None, :]).astype(
        np.float32)
    ones1 = np.ones((1, P), np.float16)
    in_maps = []
    for c in range(N_CORES):
        in_maps.append({
            "pose": flat[c * BN_PER_CORE:(c + 1) * BN_PER_CORE],
            "codebook": cbf,
            "iota4": iota4,
            "ones1": ones1,
            "ident": np.eye(P, dtype=np.float32),
        })
    return in_maps


def finish_host(dots, accs) -> np.ndarray:
    """Cross-core unshard: PE diag blocks + gathered slot sums -> loss."""
    S = np.zeros((G8, G8), dtype=np.float64)
    for d in dots:
        d4 = np.asarray(d, dtype=np.float64).reshape(16, G8, 16, G8)
        S += np.einsum("jgjh->gh", d4)
    slot = np.zeros(NPAIR, dtype=np.float64)
    for a in accs:
        a4 = np.asarray(a, dtype=np.float64).reshape(GC, CH, NPAIR, P)
        slot += a4.sum(axis=(0, 1, 3))
    for s, (g, h) in enumerate(PAIRS):
        S[g, h] += slot[s]
    m = S / float(BN)
    iu = np.triu_indices(G8, k=1)
    loss = 2.0 * np.abs(m[iu]).sum()
    return np.float32(loss)


def kernel(pose_code: np.ndarray, codebook: np.ndarray) -> np.ndarray:
    nc = _get_nc()
    in_maps = make_in_maps(pose_code, codebook)
    res = run_bass_kernel_spmd(nc, in_maps, core_ids=list(range(N_CORES)))
    loss = finish_host([res.results[c]["dot_out"] for c in range(N_CORES)],
                       [res.results[c]["acc_out"] for c in range(N_CORES)])
    return loss.reshape(()).astype(np.float32)
